# revision 2
# baseline (speedup 1.0000x reference)
"""Binary-weight dense layer on 8 trn2 NeuronCores.

Computes out[b,s,f] = scale * sum_i x[b,s,i] * (kernel[i,f] ? +1 : -1)
for x [4, 4096, 1024] f32, kernel [1024, 1024] bool, scale scalar f32.

Strategy: data-parallel over the 16384 rows (2048 rows/core).  Host-side
prep transposes each x shard to [K, rows] bf16 (scale folded into the
+-1 weights, exact in bf16 for power-of-two scales) and repacks it into
two k-interleaved DRAM blobs so every DMA moves >=2KB contiguous lines.
On-chip: bf16 matmul accumulating fp32 in PSUM, DVE copy (with bf16
downcast) to SBUF, DMA out; host upconverts the bf16 result to f32.

Schedule: no warmup matmuls -- the first real k-chunk runs inside the
HAM cold window while the rest of the inputs stream in.  Phase 1 does
m-tiles 0..3 k-major (consuming chunks as they land), phase 2 runs
m-major with PSUM-bank recycling and per-tile eviction overlapped with
the next tile's matmuls.  Inputs are split across the two HWDGE rings
(sync: x, scalar: w) in consumption order; outputs alternate rings and
the last tile is evicted in halves to shorten the tail.
"""

import numpy as np
import ml_dtypes

import concourse.bacc as bacc
import concourse.mybir as mybir
import concourse.tile as tile
from concourse.bass_utils import run_bass_kernel_spmd

N_CORES = 8
B, S, K, N = 4, 4096, 1024, 1024
ROWS = B * S                    # 16384
ROWS_PER_CORE = ROWS // N_CORES  # 2048
P = 128                         # partitions
KT = K // P                     # 8 contraction subtiles
MT = ROWS_PER_CORE // P         # 16 row tiles per core
NHALF = 512                     # one PSUM bank of f32
G0 = 4                          # m-tiles processed k-major during load phase
ACOLS = G0 * P                  # 512 leading row-columns (phase-1 x)
BCOLS = ROWS_PER_CORE - ACOLS   # 1536 trailing row-columns (phase-2 x)

_module_cache = {}


def build_module():
    nc = bacc.Bacc(None)
    # xa[p, k*ACOLS + c] = x^T[k*P + p, c]          (rows 0..512 of the shard)
    # xb[p, k*BCOLS + c] = x^T[k*P + p, ACOLS + c]  (rows 512..2048)
    xa = nc.dram_tensor("xa", [P, KT * ACOLS], mybir.dt.bfloat16,
                        kind="ExternalInput")
    xb = nc.dram_tensor("xb", [P, KT * BCOLS], mybir.dt.bfloat16,
                        kind="ExternalInput")
    w = nc.dram_tensor("w", [K, N], mybir.dt.bfloat16, kind="ExternalInput")
    out = nc.dram_tensor("out", [ROWS_PER_CORE, N], mybir.dt.bfloat16,
                         kind="ExternalOutput")

    with tile.TileContext(nc) as tc:
        with (
            tc.tile_pool(name="persist", bufs=1) as persist,
            tc.tile_pool(name="psum", bufs=1, space="PSUM") as ps_pool,
            tc.tile_pool(name="outp", bufs=3) as out_pool,
        ):
            # --- input DMAs, one tile per DMA so buffer-level dependency
            # tracking never over-serializes.  Emission order per engine is
            # consumption order; every transfer's source is ready at t=0 so
            # the rings stream back-to-back.
            # sync ring: x (1 MB + 3 MB), scalar ring: w (2 MB).
            xa_t = [None] * (KT // 2)
            for j in range(KT // 2):      # k-pairs 01, 23, 45, 67
                t = persist.tile([P, 2 * ACOLS], mybir.dt.bfloat16,
                                 tag=f"xa{j}")
                nc.sync.dma_start(out=t,
                                  in_=xa[:, 2 * j * ACOLS:(2 * j + 2) * ACOLS])
                xa_t[j] = t

            w_t = [None] * KT
            w0a = persist.tile([P, NHALF], mybir.dt.bfloat16, tag="w0a")
            nc.scalar.dma_start(out=w0a, in_=w[0:P, 0:NHALF])
            w0b = persist.tile([P, NHALF], mybir.dt.bfloat16, tag="w0b")
            nc.scalar.dma_start(out=w0b, in_=w[0:P, NHALF:N])
            for k in range(1, KT):
                t = persist.tile([P, N], mybir.dt.bfloat16, tag=f"w{k}")
                nc.scalar.dma_start(out=t, in_=w[k * P:(k + 1) * P, :])
                w_t[k] = t

            xb_t = [None] * 2
            for j in range(2):            # k-quads 0123, 4567
                t = persist.tile([P, 4 * BCOLS], mybir.dt.bfloat16,
                                 tag=f"xb{j}")
                ring = nc.sync if j == 0 else nc.scalar
                ring.dma_start(out=t,
                               in_=xb[:, 4 * j * BCOLS:(4 * j + 4) * BCOLS])
                xb_t[j] = t

            def lhsT(m, k):
                if m < G0:
                    t = xa_t[k // 2]
                    off = (k % 2) * ACOLS + m * P
                else:
                    t = xb_t[k // 4]
                    off = (k % 4) * BCOLS + (m - G0) * P
                return t[:, off:off + P]

            def rhs(k, h):
                if k == 0:
                    return (w0a if h == 0 else w0b)[:, 0:NHALF]
                return w_t[k][:, h * NHALF:(h + 1) * NHALF]

            ps_tiles = {}

            def mm(m, k):
                ps = ps_tiles[m % G0]
                lt = lhsT(m, k)
                nc.tensor.matmul(ps[:, 0:NHALF], lt, rhs(k, 0),
                                 start=(k == 0), stop=(k == KT - 1))
                nc.tensor.matmul(ps[:, NHALF:N], lt, rhs(k, 1),
                                 start=(k == 0), stop=(k == KT - 1))

            def evict(m):
                ot = out_pool.tile([P, N], mybir.dt.bfloat16, tag="ot")
                ring = nc.sync if m % 2 == 0 else nc.scalar
                if m == MT - 1:
                    # last tile: half copies + stores on both rings so the
                    # first half's transfer overlaps the second's copy
                    nc.vector.tensor_copy(ot[:, 0:NHALF],
                                          ps_tiles[m % G0][:, 0:NHALF])
                    nc.scalar.dma_start(out=out[m * P:(m + 1) * P, 0:NHALF],
                                        in_=ot[:, 0:NHALF])
                    nc.vector.tensor_copy(ot[:, NHALF:N],
                                          ps_tiles[m % G0][:, NHALF:N])
                    nc.sync.dma_start(out=out[m * P:(m + 1) * P, NHALF:N],
                                      in_=ot[:, NHALF:N])
                else:
                    nc.vector.tensor_copy(ot, ps_tiles[m % G0])
                    ring.dma_start(out=out[m * P:(m + 1) * P, :], in_=ot)

            # Phase 1: first G0 m-tiles k-major, consuming chunks as they
            # arrive from DMA.  k=0 runs inside the HAM cold window.
            for m in range(G0):
                ps_tiles[m] = ps_pool.tile([P, N], mybir.dt.float32,
                                           tag=f"ps{m}", name=f"ps{m}")
            for k in range(KT):
                for m in range(G0):
                    mm(m, k)
            for m in range(G0):
                evict(m)

            # Phase 2: remaining m-tiles m-major (inputs now resident),
            # copy-out pipelined with the next tile's matmuls.
            for m in range(G0, MT):
                ps_tiles[m % G0] = ps_pool.tile([P, N], mybir.dt.float32,
                                                tag=f"ps{m % G0}",
                                                name=f"ps{m}")
                for k in range(KT):
                    mm(m, k)
                evict(m)
    nc.finalize()
    return nc


def get_module():
    if "nc" not in _module_cache:
        _module_cache["nc"] = build_module()
    return _module_cache["nc"]


def _prepare_in_maps(x, kernel, scale):
    bf16 = ml_dtypes.bfloat16
    x2d = np.asarray(x, dtype=np.float32).reshape(ROWS, K)
    scale = np.float32(scale)
    w_signed = np.where(np.asarray(kernel, dtype=bool), scale, -scale)
    w_bf16 = np.ascontiguousarray(w_signed.astype(bf16))
    in_maps = []
    for c in range(N_CORES):
        shard = x2d[c * ROWS_PER_CORE:(c + 1) * ROWS_PER_CORE]
        xt = shard.T.astype(bf16)                     # [K, rows]
        xt3 = xt.reshape(KT, P, ROWS_PER_CORE)        # [k, p, rows]
        xa_c = np.ascontiguousarray(
            xt3[:, :, 0:ACOLS].transpose(1, 0, 2)).reshape(P, KT * ACOLS)
        xb_c = np.ascontiguousarray(
            xt3[:, :, ACOLS:].transpose(1, 0, 2)).reshape(P, KT * BCOLS)
        in_maps.append({"xa": xa_c, "xb": xb_c, "w": w_bf16})
    return in_maps


def kernel(x, kernel, scale):
    nc = get_module()
    in_maps = _prepare_in_maps(x, kernel, scale)
    res = run_bass_kernel_spmd(nc, in_maps, core_ids=list(range(N_CORES)))
    out = np.concatenate(
        [r["out"].astype(np.float32) for r in res.results], axis=0)
    return out.reshape(B, S, N)
